# revision 34
# baseline (speedup 1.0000x reference)
"""Causal self-attention (QK-RMSNorm + rotary, H=16, D=1024, B=2, T=2048) on 8 NeuronCores.

Sharding: core c handles batch b = c // 4 and heads 4*(c%4) .. 4*(c%4)+3.
Each core computes the qkv projection for its heads, causal attention, and a
row-parallel slice of the output projection; the host sums the 4 partial
outputs per batch element.

All matmul operands are bf16 (fp32 PSUM accumulation): fast-weight-load and
half the DMA traffic. The qk projection runs weight-stationary (mt/k outer,
chunks inner; one accumulator bank per chunk) so each weight tile loads
once. Rotary is one 128x128 sign-permutation matmul per projection tile
(rot(q) = P q). Scores contract K=64 per head with the two heads of a pair
row-tiled concurrently into the PE array (partitions 0-63 / 64-127); one
exp activation covers both heads' trimmed score regions via a 2-D access
pattern. Softmax sums are gathered by selector matmuls (no DMA), divided
per i-block, and the output projection runs per i-block so it overlaps the
next block's attention; i-blocks run largest-first so the tail is short.
Softmax needs no running max: RMS-normalized q,k bound scores to
|s| <= sqrt(d_head) * ||q|| = 8.
"""
import sys
sys.path.insert(0, '/opt/trn_rl_repo')

import numpy as np
import ml_dtypes
from contextlib import ExitStack

import concourse.bass as bass
import concourse.tile as tile
from concourse import bacc, mybir
from concourse.bass_utils import run_bass_kernel_spmd

F32 = mybir.dt.float32
BF16 = mybir.dt.bfloat16
AF = mybir.ActivationFunctionType

N_HEAD = 16
D_MODEL = 1024
D_HEAD = 64
B, T = 2, 2048
N_CORES = 8
HL = 4            # heads per core (2 pairs)
KT = D_MODEL // 128   # 8 contraction tiles
NCH = T // 512    # 4 chunks (projection)
NIB = T // 512    # 4 i-blocks (attention)
NTT = T // 128    # 16 token tiles
SCALE = D_HEAD ** -0.5

_cached = {}


def _build():
    nc = bacc.Bacc("TRN2", target_bir_lowering=False, debug=False,
                   num_devices=N_CORES)

    # ---- DRAM I/O ----------------------------------------------------------
    xT = nc.dram_tensor("xT", [D_MODEL, T], BF16, kind="ExternalInput").ap()
    wqk = nc.dram_tensor("wqk", [D_MODEL, 512], BF16, kind="ExternalInput").ap()
    wv = nc.dram_tensor("wv", [D_MODEL, 256], BF16, kind="ExternalInput").ap()
    wpP = nc.dram_tensor("wpP", [2, 128, 1024], BF16, kind="ExternalInput").ap()
    cosT = nc.dram_tensor("cosT", [128, T], BF16, kind="ExternalInput").ap()
    sinT = nc.dram_tensor("sinT", [128, T], BF16, kind="ExternalInput").ap()
    permT = nc.dram_tensor("permT", [128, 128], BF16, kind="ExternalInput").ap()
    trimask = nc.dram_tensor("trimask", [128, 128], BF16,
                             kind="ExternalInput").ap()
    rsel32 = nc.dram_tensor("rsel32", [128, 16 * 32], BF16,
                            kind="ExternalInput").ap()
    rselT32 = nc.dram_tensor("rselT32", [32, 16 * 128], BF16,
                             kind="ExternalInput").ap()
    selS = nc.dram_tensor("selS", [128, 128], BF16, kind="ExternalInput").ap()
    onescol = nc.dram_tensor("onescol", [128, HL * NTT], BF16,
                             kind="ExternalInput").ap()
    out = nc.dram_tensor("out", [T, D_MODEL], BF16,
                         kind="ExternalOutput").ap()

    with tile.TileContext(nc) as tc, ExitStack() as ctx:
        ctx.enter_context(nc.allow_low_precision(
            reason="bf16 matmul operands / bf16 elementwise; fp32 PSUM"))

        cpool = ctx.enter_context(tc.tile_pool(name="consts", bufs=1))
        ppool = ctx.enter_context(tc.tile_pool(name="persist", bufs=1))

        # persistent activations (pair layout: even head on partitions 0-63,
        # odd head on 64-127)
        qT_sb = [ppool.tile([128, T], BF16, tag=f"qT{p}", name=f"qT{p}")
                 for p in range(2)]
        kT_sb = [ppool.tile([128, T], BF16, tag=f"kT{p}", name=f"kT{p}")
                 for p in range(2)]
        v_sb = ppool.tile([128, HL * NTT * 65 + 64], BF16, tag="v")
        v4 = v_sb[:, 0:HL * NTT * 65].rearrange("p (h t o) -> p h t o",
                                                h=HL, o=65)
        yP = [ppool.tile([128, T], BF16, tag=f"yP{t}", name=f"yP{t}")
              for t in range(2)]

        perm_sb = cpool.tile([128, 128], BF16)
        tri_sb = cpool.tile([128, 128], BF16)
        rsel32_sb = cpool.tile([128, 16 * 32], BF16)
        rselT32_sb = cpool.tile([32, 16 * 128], BF16)
        selS_sb = cpool.tile([128, 128], BF16)
        wpP_sb = [cpool.tile([128, 1024], BF16, tag=f"wpP{t}", name=f"wpP{t}")
                  for t in range(2)]

        # ---- phase 1: projections + rope + rmsnorm + v ---------------------
        with tc.tile_pool(name="wts", bufs=1) as wtp, \
             tc.tile_pool(name="pwork", bufs=3) as wpool, \
             tc.tile_pool(name="sqw", bufs=3) as sqpool, \
             tc.tile_pool(name="psqk", bufs=1, space="PSUM") as ps_qk, \
             tc.tile_pool(name="psrot", bufs=1, space="PSUM") as ps_rot, \
             tc.tile_pool(name="psstat", bufs=1, space="PSUM") as ps_stat, \
             tc.tile_pool(name="psbc", bufs=1, space="PSUM") as ps_bc:
            wqk_sb = [wtp.tile([128, 512], BF16, tag=f"wqk{k}",
                               name=f"wqk{k}") for k in range(KT)]
            wv_sb = [wtp.tile([128, 256], BF16, tag=f"wv{k}", name=f"wv{k}")
                     for k in range(KT)]
            xt = [wtp.tile([128, T], BF16, tag=f"xt{k}", name=f"xt{k}")
                  for k in range(KT)]
            # weights on the sync queue, x rows on the scalar DGE queue,
            # k-major to match consumption order (one 1MB transfer per k)
            for k in range(KT):
                ks = slice(k * 128, (k + 1) * 128)
                nc.sync.dma_start(wqk_sb[k][:], wqk[ks, :])
                if k == 0:  # split so the very first matmul can start sooner
                    nc.scalar.dma_start(xt[0][:, 0:512], xT[ks, 0:512])
                    nc.scalar.dma_start(xt[0][:, 512:T], xT[ks, 512:T])
                else:
                    nc.scalar.dma_start(xt[k][:], xT[ks, :])
            nc.sync.dma_start(perm_sb[:], permT[:])
            cos_sb = wtp.tile([128, T], BF16)
            nc.sync.dma_start(cos_sb[:], cosT[:])
            sin_sb = wtp.tile([128, T], BF16)
            nc.sync.dma_start(sin_sb[:], sinT[:])
            nc.sync.dma_start(rsel32_sb[:], rsel32[:])
            nc.sync.dma_start(rselT32_sb[:], rselT32[:])
            for k in range(KT):
                ks = slice(k * 128, (k + 1) * 128)
                nc.sync.dma_start(wv_sb[k][:], wv[ks, :])
            nc.sync.dma_start(tri_sb[:], trimask[:])
            nc.sync.dma_start(selS_sb[:], selS[:])
            for t in range(2):
                nc.sync.dma_start(wpP_sb[t][:], wpP[t])
            nc.sync.dma_start(v4[:, :, :, 64:65],
                              onescol.rearrange("p (h t) -> p h t",
                                                h=HL).unsqueeze(3))

            ssq_all = ps_stat.tile([32, 512], F32, tag="stat")
            dsts = {}
            # weight-stationary qk: mt/k outer, chunks inner
            for mt in range(4):
                accs = [ps_qk.tile([128, 512], F32, tag=f"qk{ch}",
                                   name=f"acc{mt}_{ch}")
                        for ch in range(NCH)]
                for k in range(KT):
                    for ch in range(NCH):
                        nc.tensor.matmul(accs[ch][:],
                                         wqk_sb[k][:,
                                                   mt * 128:(mt + 1) * 128],
                                         xt[k][:, ch * 512:ch * 512 + 512],
                                         start=(k == 0),
                                         stop=(k == KT - 1))
                # evacuate all four accumulators first so the next mt's
                # matmuls get their PSUM banks back promptly
                s_ts = []
                for ch in range(NCH):
                    s_t = wpool.tile([128, 512], BF16, tag=f"s{ch}",
                                     name=f"s{mt}_{ch}")
                    nc.scalar.copy(s_t[:], accs[ch][:])
                    s_ts.append(s_t)
                for ch in range(NCH):
                    cs = ch * 512
                    s_t = s_ts[ch]
                    accr = ps_rot.tile([128, 512], F32, tag="rot")
                    nc.tensor.matmul(accr[:], perm_sb[:], s_t[:],
                                     start=True, stop=True)
                    sr_t = wpool.tile([128, 512], BF16, tag="sr")
                    nc.vector.tensor_copy(sr_t[:], accr[:])
                    t1 = wpool.tile([128, 512], BF16, tag="t1")
                    nc.vector.tensor_mul(t1[:], s_t[:], cos_sb[:, cs:cs + 512])
                    t2 = wpool.tile([128, 512], BF16, tag="t2")
                    nc.vector.tensor_mul(t2[:], sr_t[:],
                                         sin_sb[:, cs:cs + 512])
                    if mt < 2:
                        d = qT_sb[mt][:, cs:cs + 512]
                    else:
                        d = kT_sb[mt - 2][:, cs:cs + 512]
                    nc.vector.tensor_add(d, t1[:], t2[:])
                    dsts[(ch, mt)] = d
                    # rotation preserves per-head norms: square the pre-rope
                    # values so the stats chain doesn't wait on the rope
                    sq = sqpool.tile([128, 512], BF16, tag="sq")
                    nc.gpsimd.tensor_mul(sq[:], s_t[:], s_t[:])
                    idx = ch * 4 + mt
                    nc.tensor.matmul(ssq_all[:],
                                     rsel32_sb[:, idx * 32:(idx + 1) * 32],
                                     sq[:], start=(idx == 0),
                                     stop=(idx == 15))

            # batched rsqrt = exp(-0.5 ln(ms)); bc/apply hides under v MMs
            lnv = wpool.tile([32, 512], F32, tag="t1")
            nc.scalar.activation(lnv[:], ssq_all[:], AF.Ln, scale=1.0 / 64.0)
            rms_all = wpool.tile([32, 512], BF16, tag="t2")
            nc.scalar.activation(rms_all[:], lnv[:], AF.Exp, scale=-0.5)
            for ch in range(NCH):
                for mt in range(4):
                    idx = ch * 4 + mt
                    bc = ps_bc.tile([128, 512], F32, tag="bc")
                    nc.tensor.matmul(bc[:],
                                     rselT32_sb[:, idx * 128:(idx + 1) * 128],
                                     rms_all[:], start=True, stop=True)
                    nc.vector.tensor_mul(dsts[(ch, mt)], dsts[(ch, mt)],
                                         bc[:])

            # v: token-major directly (lhsT = x tile, rhs = wv)
            for ch in range(NCH):
                for j in range(4):
                    tt = ch * 4 + j
                    accv = ps_rot.tile([128, 256], F32, tag="vacc")
                    for k in range(KT):
                        nc.tensor.matmul(accv[:],
                                         xt[k][:, tt * 128:(tt + 1) * 128],
                                         wv_sb[k][:], start=(k == 0),
                                         stop=(k == KT - 1))
                    av = accv[:].rearrange("p (h d) -> p h d", h=HL)
                    nc.vector.tensor_copy(v4[:, :, tt, 0:64], av)

        # ---- phase 2: attention + per-i-block softmax div + out proj -------
        with tc.tile_pool(name="pexp", bufs=4) as epool, \
             tc.tile_pool(name="ysg", bufs=2) as ysgp, \
             tc.tile_pool(name="awork", bufs=3) as awork, \
             tc.tile_pool(name="psg", bufs=2, space="PSUM") as ps_g, \
             tc.tile_pool(name="psy", bufs=1, space="PSUM") as ps_y, \
             tc.tile_pool(name="psd", bufs=1, space="PSUM") as ps_d:
            for ib in reversed(range(NIB)):
                ibs = ib * 512
                njt = 4 * (ib + 1)
                for p in range(2):
                    yaccA = ps_y.tile([128, 512], F32, tag="yA")
                    yaccB = ps_y.tile([128, 512], F32, tag="yB")
                    for jt in range(njt):
                        o = max(0, jt * 128 - ibs)
                        js = slice(jt * 128, (jt + 1) * 128)
                        qs = slice(ibs + o, ibs + 512)
                        psG = ps_g.tile([128, 1024], F32, tag="sc")
                        nc.tensor.matmul(psG[:, o:512], kT_sb[p][0:64, js],
                                         qT_sb[p][0:64, qs],
                                         start=True, stop=True)
                        nc.tensor.matmul(psG[:, 512 + o:1024],
                                         kT_sb[p][64:128, js],
                                         qT_sb[p][64:128, qs],
                                         start=True, stop=True)
                        p_sb = epool.tile([128, 1024], BF16, tag="p")
                        # one exp for both heads over the valid regions
                        src2 = psG[:].rearrange("p (h q) -> p h q", h=2)
                        dst2 = p_sb[:].rearrange("p (h q) -> p h q", h=2)
                        nc.scalar.activation(dst2[:, :, o:512],
                                             src2[:, :, o:512], AF.Exp,
                                             scale=SCALE)
                        if jt * 128 >= ibs:  # diagonal tile: triangular mask
                            nc.vector.tensor_mul(p_sb[:, o:o + 128],
                                                 p_sb[:, o:o + 128], tri_sb[:])
                            nc.vector.tensor_mul(p_sb[:, 512 + o:640 + o],
                                                 p_sb[:, 512 + o:640 + o],
                                                 tri_sb[:])
                        ha, hb = 2 * p, 2 * p + 1
                        ga, gb = ha * NTT + jt, hb * NTT + jt
                        vauA = v_sb[:, ga * 65:ga * 65 + 128]  # y@0-63, sum@64
                        vauB = v_sb[:, gb * 65 - 64:gb * 65 + 64]  # sum@63
                        nc.tensor.matmul(yaccA[:, o:512], vauA,
                                         p_sb[:, o:512],
                                         start=(jt == 0), stop=(jt == njt - 1))
                        nc.tensor.matmul(yaccB[:, o:512], vauB,
                                         p_sb[:, 512 + o:1024],
                                         start=(jt == 0), stop=(jt == njt - 1))
                    ysgA = ysgp.tile([128, 512], BF16, tag="ysgA")
                    nc.vector.tensor_copy(ysgA[:], yaccA[:])
                    ysgB = ysgp.tile([128, 512], BF16, tag="ysgB")
                    nc.vector.tensor_copy(ysgB[:], yaccB[:])

                    # softmax division: gather sums rows by selector matmul,
                    # reciprocal, scale
                    bcs = ps_d.tile([128, 512], F32, tag="bcs")
                    nc.tensor.matmul(bcs[0:64, :], selS_sb[:, 0:64],
                                     ysgA[:], start=True, stop=True)
                    nc.tensor.matmul(bcs[64:128, :], selS_sb[:, 64:128],
                                     ysgB[:], start=True, stop=True)
                    recb = awork.tile([128, 512], F32, tag="recb")
                    nc.vector.reciprocal_approx_fast(recb[:], bcs[:])
                    rec16 = awork.tile([128, 512], BF16, tag="rec16")
                    nc.vector.tensor_copy(rec16[:], recb[:])
                    nc.vector.tensor_mul(yP[p][0:64, ibs:ibs + 512],
                                         ysgA[0:64, :], rec16[0:64, :])
                    nc.vector.tensor_mul(yP[p][64:128, ibs:ibs + 512],
                                         ysgB[64:128, :], rec16[64:128, :])

                # out projection for this i-block (overlaps next block)
                for mt in range(4 * ib, 4 * ib + 4):
                    ms = slice(mt * 128, (mt + 1) * 128)
                    o_sb = awork.tile([128, 1024], BF16, tag="osb")
                    for oc in range(2):
                        acc = ps_d.tile([128, 512], F32, tag="o")
                        for t in range(2):
                            nc.tensor.matmul(acc[:], yP[t][:, ms],
                                             wpP_sb[t][:,
                                                       oc * 512:(oc + 1) * 512],
                                             start=(t == 0), stop=(t == 1))
                        nc.vector.tensor_copy(
                            o_sb[:, oc * 512:(oc + 1) * 512], acc[:])
                    # the last i-block's stores go on the (idle) scalar queue
                    eng = nc.scalar if ib == 0 else nc.sync
                    eng.dma_start(out[ms, :], o_sb[:])

    nc.compile()
    return nc


def _host_inputs(x, w_attn, w_proj):
    """Build the 8 per-core input maps."""
    bf = ml_dtypes.bfloat16
    inv_freq = 1.0 / (10000.0 ** (np.arange(0, D_HEAD, 2, dtype=np.float32)
                                  / D_HEAD))
    t = np.arange(T, dtype=np.float32)
    freqs = np.einsum('i,j->ij', t, inv_freq)          # [T, 32]
    cos64 = np.cos(np.concatenate([freqs, freqs], 1)).T  # [64, T]
    sin64 = np.sin(np.concatenate([freqs, freqs], 1)).T
    cosT = np.concatenate([cos64, cos64], 0).astype(bf)  # [128, T]
    sinT = np.concatenate([sin64, sin64], 0).astype(bf)

    # rot matrix: accr = M @ S with M[d,e] = -1 if e==d+32 (d%64<32),
    # +1 if e==d-32 (d%64>=32); matmul computes lhsT.T @ rhs -> pass M.T
    M = np.zeros((128, 128), np.float32)
    for d in range(128):
        base = (d // 64) * 64
        r = d % 64
        if r < 32:
            M[d, base + r + 32] = -1.0
        else:
            M[d, base + r - 32] = 1.0
    permT = M.T.astype(bf).copy()

    tri = (np.arange(128)[:, None] <= np.arange(128)[None, :]).astype(bf)
    rsel32 = np.zeros((128, 16 * 32), np.float32)
    rselT32 = np.zeros((32, 16 * 128), np.float32)
    for chm in range(16):
        ch, mt = chm // 4, chm % 4
        for half in range(2):
            r = ch * 8 + mt * 2 + half
            ps = slice(half * 64, half * 64 + 64)
            rsel32[ps, chm * 32 + r] = 1.0
            rselT32[r, chm * 128 + half * 64:chm * 128 + half * 64 + 64] = 1.0
    # selS: cols 0-63 pick partition 64 (even-head sums row), cols 64-127
    # pick partition 63 (odd-head sums row)
    selS = np.zeros((128, 128), np.float32)
    selS[64, 0:64] = 1.0
    selS[63, 64:128] = 1.0
    onescol = np.ones((128, HL * NTT), np.float32)

    wq = w_attn[:D_MODEL]          # [1024, 1024] rows: head h = 64h..64h+63
    wk = w_attn[D_MODEL:2 * D_MODEL]
    wv_full = w_attn[2 * D_MODEL:]

    in_maps = []
    for c in range(N_CORES):
        b, hg = c // 4, c % 4
        hs = slice(hg * 4 * D_HEAD, (hg * 4 + 4) * D_HEAD)   # 256 rows
        wqk_c = np.concatenate([wq[hs], wk[hs]], 0).T        # [1024, 512]
        wv_c = wv_full[hs].T                                 # [1024, 256]
        wp_c = [w_proj[:, (hg * 4 + j) * D_HEAD:(hg * 4 + j + 1) * D_HEAD].T
                for j in range(HL)]                          # 4x[64,1024]
        wpP_c = np.stack([np.concatenate([wp_c[0], wp_c[1]], 0),
                          np.concatenate([wp_c[2], wp_c[3]], 0)])
        in_maps.append({
            "xT": np.ascontiguousarray(x[b].T.astype(bf)),
            "wqk": np.ascontiguousarray(wqk_c.astype(bf)),
            "wv": np.ascontiguousarray(wv_c.astype(bf)),
            "wpP": np.ascontiguousarray(wpP_c.astype(bf)),
            "cosT": cosT, "sinT": sinT, "permT": permT, "trimask": tri,
            "rsel32": rsel32.astype(bf), "rselT32": rselT32.astype(bf),
            "selS": selS.astype(bf), "onescol": onescol.astype(bf),
        })
    return in_maps


def kernel(x, w_attn, w_proj, _want_results=False):
    x = np.asarray(x, dtype=np.float32)
    w_attn = np.asarray(w_attn, dtype=np.float32)
    w_proj = np.asarray(w_proj, dtype=np.float32)

    if "nc" not in _cached:
        _cached["nc"] = _build()
    nc = _cached["nc"]

    in_maps = _host_inputs(x, w_attn, w_proj)
    res = run_bass_kernel_spmd(nc, in_maps, list(range(N_CORES)))

    full = np.zeros((B, T, D_MODEL), np.float32)
    for c in range(N_CORES):
        full[c // 4] += np.asarray(res.results[c]["out"], dtype=np.float32)
    if _want_results:
        return full, res
    return full


# revision 36
# speedup vs baseline: 1.1712x; 1.1712x over previous
"""Causal self-attention (QK-RMSNorm + rotary, H=16, D=1024, B=2, T=2048) on 8 NeuronCores.

Sharding: core c handles batch b = c // 4 and heads 4*(c%4) .. 4*(c%4)+3.
Each core computes the qkv projection for its heads, causal attention, and a
row-parallel slice of the output projection; the host sums the 4 partial
outputs per batch element.

All matmul operands are bf16 (fp32 PSUM accumulation): fast-weight-load and
half the DMA traffic. The qk projection runs weight-stationary (mt/k outer,
chunks inner; one accumulator bank per chunk) so each weight tile loads
once. Rotary is one 128x128 sign-permutation matmul per projection tile
(rot(q) = P q). Scores contract K=64 per head with the two heads of a pair
row-tiled concurrently into the PE array (partitions 0-63 / 64-127); one
exp activation covers both heads' trimmed score regions via a 2-D access
pattern. Softmax sums are gathered by selector matmuls (no DMA), divided
per i-block, and the output projection runs per i-block so it overlaps the
next block's attention; i-blocks run largest-first so the tail is short.
Softmax needs no running max: RMS-normalized q,k bound scores to
|s| <= sqrt(d_head) * ||q|| = 8.
"""
import sys
sys.path.insert(0, '/opt/trn_rl_repo')

import numpy as np
import ml_dtypes
from contextlib import ExitStack

import concourse.bass as bass
import concourse.tile as tile
from concourse import bacc, mybir
from concourse.bass_utils import run_bass_kernel_spmd

F32 = mybir.dt.float32
BF16 = mybir.dt.bfloat16
AF = mybir.ActivationFunctionType

N_HEAD = 16
D_MODEL = 1024
D_HEAD = 64
B, T = 2, 2048
N_CORES = 8
HL = 4            # heads per core (2 pairs)
KT = D_MODEL // 128   # 8 contraction tiles
NCH = T // 512    # 4 chunks (projection)
NIB = T // 512    # 4 i-blocks (attention)
NTT = T // 128    # 16 token tiles
SCALE = D_HEAD ** -0.5

_cached = {}


def _build():
    nc = bacc.Bacc("TRN2", target_bir_lowering=False, debug=False,
                   num_devices=N_CORES)

    # ---- DRAM I/O ----------------------------------------------------------
    xT = nc.dram_tensor("xT", [D_MODEL, T], BF16, kind="ExternalInput").ap()
    wqk = nc.dram_tensor("wqk", [D_MODEL, 512], BF16, kind="ExternalInput").ap()
    wv = nc.dram_tensor("wv", [D_MODEL, 256], BF16, kind="ExternalInput").ap()
    wpP = nc.dram_tensor("wpP", [2, 128, 1024], BF16, kind="ExternalInput").ap()
    cosT = nc.dram_tensor("cosT", [128, T], BF16, kind="ExternalInput").ap()
    sinT = nc.dram_tensor("sinT", [128, T], BF16, kind="ExternalInput").ap()
    permT = nc.dram_tensor("permT", [128, 128], BF16, kind="ExternalInput").ap()
    trimask = nc.dram_tensor("trimask", [128, 128], BF16,
                             kind="ExternalInput").ap()
    rsel32 = nc.dram_tensor("rsel32", [128, 16 * 32], BF16,
                            kind="ExternalInput").ap()
    rselT32 = nc.dram_tensor("rselT32", [32, 16 * 128], BF16,
                             kind="ExternalInput").ap()
    selS = nc.dram_tensor("selS", [128, 128], BF16, kind="ExternalInput").ap()
    onescol = nc.dram_tensor("onescol", [128, HL * NTT], BF16,
                             kind="ExternalInput").ap()
    out = nc.dram_tensor("out", [T, D_MODEL], F32, kind="ExternalOutput").ap()

    with tile.TileContext(nc) as tc, ExitStack() as ctx:
        ctx.enter_context(nc.allow_low_precision(
            reason="bf16 matmul operands / bf16 elementwise; fp32 PSUM"))

        cpool = ctx.enter_context(tc.tile_pool(name="consts", bufs=1))
        ppool = ctx.enter_context(tc.tile_pool(name="persist", bufs=1))

        # persistent activations (pair layout: even head on partitions 0-63,
        # odd head on 64-127)
        qT_sb = [ppool.tile([128, T], BF16, tag=f"qT{p}", name=f"qT{p}")
                 for p in range(2)]
        kT_sb = [ppool.tile([128, T], BF16, tag=f"kT{p}", name=f"kT{p}")
                 for p in range(2)]
        v_sb = ppool.tile([128, HL * NTT * 65 + 64], BF16, tag="v")
        v4 = v_sb[:, 0:HL * NTT * 65].rearrange("p (h t o) -> p h t o",
                                                h=HL, o=65)
        yP = [ppool.tile([128, T], BF16, tag=f"yP{t}", name=f"yP{t}")
              for t in range(2)]

        perm_sb = cpool.tile([128, 128], BF16)
        tri_sb = cpool.tile([128, 128], BF16)
        rsel32_sb = cpool.tile([128, 16 * 32], BF16)
        rselT32_sb = cpool.tile([32, 16 * 128], BF16)
        selS_sb = cpool.tile([128, 128], BF16)
        wpP_sb = [cpool.tile([128, 1024], BF16, tag=f"wpP{t}", name=f"wpP{t}")
                  for t in range(2)]

        # ---- phase 1: projections + rope + rmsnorm + v ---------------------
        with tc.tile_pool(name="wts", bufs=1) as wtp, \
             tc.tile_pool(name="pwork", bufs=3) as wpool, \
             tc.tile_pool(name="sqw", bufs=3) as sqpool, \
             tc.tile_pool(name="psqk", bufs=1, space="PSUM") as ps_qk, \
             tc.tile_pool(name="psrot", bufs=1, space="PSUM") as ps_rot, \
             tc.tile_pool(name="psstat", bufs=1, space="PSUM") as ps_stat, \
             tc.tile_pool(name="psbc", bufs=1, space="PSUM") as ps_bc:
            wqk_sb = [wtp.tile([128, 512], BF16, tag=f"wqk{k}",
                               name=f"wqk{k}") for k in range(KT)]
            wv_sb = [wtp.tile([128, 256], BF16, tag=f"wv{k}", name=f"wv{k}")
                     for k in range(KT)]
            xt = [wtp.tile([128, T], BF16, tag=f"xt{k}", name=f"xt{k}")
                  for k in range(KT)]
            # weights on the sync queue, x rows on the scalar DGE queue,
            # k-major to match consumption order (one 1MB transfer per k)
            for k in range(KT):
                ks = slice(k * 128, (k + 1) * 128)
                nc.sync.dma_start(wqk_sb[k][:], wqk[ks, :])
                if k == 0:  # split so the very first matmul can start sooner
                    nc.scalar.dma_start(xt[0][:, 0:512], xT[ks, 0:512])
                    nc.scalar.dma_start(xt[0][:, 512:T], xT[ks, 512:T])
                else:
                    nc.scalar.dma_start(xt[k][:], xT[ks, :])
            nc.sync.dma_start(perm_sb[:], permT[:])
            cos_sb = wtp.tile([128, T], BF16)
            nc.sync.dma_start(cos_sb[:], cosT[:])
            sin_sb = wtp.tile([128, T], BF16)
            nc.sync.dma_start(sin_sb[:], sinT[:])
            nc.sync.dma_start(rsel32_sb[:], rsel32[:])
            nc.sync.dma_start(rselT32_sb[:], rselT32[:])
            for k in range(KT):
                ks = slice(k * 128, (k + 1) * 128)
                nc.sync.dma_start(wv_sb[k][:], wv[ks, :])
            nc.sync.dma_start(tri_sb[:], trimask[:])
            nc.sync.dma_start(selS_sb[:], selS[:])
            for t in range(2):
                nc.sync.dma_start(wpP_sb[t][:], wpP[t])
            nc.sync.dma_start(v4[:, :, :, 64:65],
                              onescol.rearrange("p (h t) -> p h t",
                                                h=HL).unsqueeze(3))

            ssq_all = ps_stat.tile([32, 512], F32, tag="stat")
            dsts = {}
            # weight-stationary qk: mt/k outer, chunks inner
            for mt in range(4):
                accs = [ps_qk.tile([128, 512], F32, tag=f"qk{ch}",
                                   name=f"acc{mt}_{ch}")
                        for ch in range(NCH)]
                for k in range(KT):
                    for ch in range(NCH):
                        nc.tensor.matmul(accs[ch][:],
                                         wqk_sb[k][:,
                                                   mt * 128:(mt + 1) * 128],
                                         xt[k][:, ch * 512:ch * 512 + 512],
                                         start=(k == 0),
                                         stop=(k == KT - 1))
                # evacuate all four accumulators first so the next mt's
                # matmuls get their PSUM banks back promptly
                s_ts = []
                for ch in range(NCH):
                    s_t = wpool.tile([128, 512], BF16, tag=f"s{ch}",
                                     name=f"s{mt}_{ch}")
                    nc.scalar.copy(s_t[:], accs[ch][:])
                    s_ts.append(s_t)
                for ch in range(NCH):
                    cs = ch * 512
                    s_t = s_ts[ch]
                    accr = ps_rot.tile([128, 512], F32, tag="rot")
                    nc.tensor.matmul(accr[:], perm_sb[:], s_t[:],
                                     start=True, stop=True)
                    sr_t = wpool.tile([128, 512], BF16, tag="sr")
                    nc.vector.tensor_copy(sr_t[:], accr[:])
                    t1 = wpool.tile([128, 512], BF16, tag="t1")
                    nc.vector.tensor_mul(t1[:], s_t[:], cos_sb[:, cs:cs + 512])
                    t2 = wpool.tile([128, 512], BF16, tag="t2")
                    nc.vector.tensor_mul(t2[:], sr_t[:],
                                         sin_sb[:, cs:cs + 512])
                    if mt < 2:
                        d = qT_sb[mt][:, cs:cs + 512]
                    else:
                        d = kT_sb[mt - 2][:, cs:cs + 512]
                    nc.vector.tensor_add(d, t1[:], t2[:])
                    dsts[(ch, mt)] = d
                    # rotation preserves per-head norms: square the pre-rope
                    # values so the stats chain doesn't wait on the rope
                    sq = sqpool.tile([128, 512], BF16, tag="sq")
                    nc.gpsimd.tensor_mul(sq[:], s_t[:], s_t[:])
                    idx = ch * 4 + mt
                    nc.tensor.matmul(ssq_all[:],
                                     rsel32_sb[:, idx * 32:(idx + 1) * 32],
                                     sq[:], start=(idx == 0),
                                     stop=(idx == 15))

            # batched rsqrt = exp(-0.5 ln(ms)); bc/apply hides under v MMs
            lnv = wpool.tile([32, 512], F32, tag="t1")
            nc.scalar.activation(lnv[:], ssq_all[:], AF.Ln, scale=1.0 / 64.0)
            rms_all = wpool.tile([32, 512], BF16, tag="t2")
            nc.scalar.activation(rms_all[:], lnv[:], AF.Exp, scale=-0.5)
            for ch in range(NCH):
                for mt in range(4):
                    idx = ch * 4 + mt
                    bc = ps_bc.tile([128, 512], F32, tag="bc")
                    nc.tensor.matmul(bc[:],
                                     rselT32_sb[:, idx * 128:(idx + 1) * 128],
                                     rms_all[:], start=True, stop=True)
                    nc.vector.tensor_mul(dsts[(ch, mt)], dsts[(ch, mt)],
                                         bc[:])

            # v: token-major directly (lhsT = x tile, rhs = wv)
            for ch in range(NCH):
                for j in range(4):
                    tt = ch * 4 + j
                    accv = ps_rot.tile([128, 256], F32, tag="vacc")
                    for k in range(KT):
                        nc.tensor.matmul(accv[:],
                                         xt[k][:, tt * 128:(tt + 1) * 128],
                                         wv_sb[k][:], start=(k == 0),
                                         stop=(k == KT - 1))
                    av = accv[:].rearrange("p (h d) -> p h d", h=HL)
                    nc.vector.tensor_copy(v4[:, :, tt, 0:64], av)

        # ---- phase 2: attention + per-i-block softmax div + out proj -------
        with tc.tile_pool(name="pexp", bufs=4) as epool, \
             tc.tile_pool(name="ysg", bufs=2) as ysgp, \
             tc.tile_pool(name="awork", bufs=3) as awork, \
             tc.tile_pool(name="psg", bufs=2, space="PSUM") as ps_g, \
             tc.tile_pool(name="psy", bufs=1, space="PSUM") as ps_y, \
             tc.tile_pool(name="psd", bufs=1, space="PSUM") as ps_d:
            for ib in reversed(range(NIB)):
                ibs = ib * 512
                njt = 4 * (ib + 1)
                for p in range(2):
                    yaccA = ps_y.tile([128, 512], F32, tag="yA")
                    yaccB = ps_y.tile([128, 512], F32, tag="yB")
                    for jt in range(njt):
                        o = max(0, jt * 128 - ibs)
                        js = slice(jt * 128, (jt + 1) * 128)
                        qs = slice(ibs + o, ibs + 512)
                        psG = ps_g.tile([128, 1024], F32, tag="sc")
                        nc.tensor.matmul(psG[:, o:512], kT_sb[p][0:64, js],
                                         qT_sb[p][0:64, qs],
                                         start=True, stop=True)
                        nc.tensor.matmul(psG[:, 512 + o:1024],
                                         kT_sb[p][64:128, js],
                                         qT_sb[p][64:128, qs],
                                         start=True, stop=True)
                        p_sb = epool.tile([128, 1024], BF16, tag="p")
                        # one exp for both heads over the valid regions
                        src2 = psG[:].rearrange("p (h q) -> p h q", h=2)
                        dst2 = p_sb[:].rearrange("p (h q) -> p h q", h=2)
                        nc.scalar.activation(dst2[:, :, o:512],
                                             src2[:, :, o:512], AF.Exp,
                                             scale=SCALE)
                        if jt * 128 >= ibs:  # diagonal tile: triangular mask
                            nc.vector.tensor_mul(p_sb[:, o:o + 128],
                                                 p_sb[:, o:o + 128], tri_sb[:])
                            nc.vector.tensor_mul(p_sb[:, 512 + o:640 + o],
                                                 p_sb[:, 512 + o:640 + o],
                                                 tri_sb[:])
                        ha, hb = 2 * p, 2 * p + 1
                        ga, gb = ha * NTT + jt, hb * NTT + jt
                        vauA = v_sb[:, ga * 65:ga * 65 + 128]  # y@0-63, sum@64
                        vauB = v_sb[:, gb * 65 - 64:gb * 65 + 64]  # sum@63
                        nc.tensor.matmul(yaccA[:, o:512], vauA,
                                         p_sb[:, o:512],
                                         start=(jt == 0), stop=(jt == njt - 1))
                        nc.tensor.matmul(yaccB[:, o:512], vauB,
                                         p_sb[:, 512 + o:1024],
                                         start=(jt == 0), stop=(jt == njt - 1))
                    ysgA = ysgp.tile([128, 512], BF16, tag="ysgA")
                    nc.vector.tensor_copy(ysgA[:], yaccA[:])
                    ysgB = ysgp.tile([128, 512], BF16, tag="ysgB")
                    nc.vector.tensor_copy(ysgB[:], yaccB[:])

                    # softmax division: gather sums rows by selector matmul,
                    # reciprocal, scale
                    bcs = ps_d.tile([128, 512], F32, tag="bcs")
                    nc.tensor.matmul(bcs[0:64, :], selS_sb[:, 0:64],
                                     ysgA[:], start=True, stop=True)
                    nc.tensor.matmul(bcs[64:128, :], selS_sb[:, 64:128],
                                     ysgB[:], start=True, stop=True)
                    recb = awork.tile([128, 512], F32, tag="recb")
                    nc.vector.reciprocal_approx_fast(recb[:], bcs[:])
                    rec16 = awork.tile([128, 512], BF16, tag="rec16")
                    nc.vector.tensor_copy(rec16[:], recb[:])
                    nc.vector.tensor_mul(yP[p][0:64, ibs:ibs + 512],
                                         ysgA[0:64, :], rec16[0:64, :])
                    nc.vector.tensor_mul(yP[p][64:128, ibs:ibs + 512],
                                         ysgB[64:128, :], rec16[64:128, :])

                # out projection for this i-block (overlaps next block)
                for mt in range(4 * ib, 4 * ib + 4):
                    ms = slice(mt * 128, (mt + 1) * 128)
                    o_sb = awork.tile([128, 1024], F32, tag="osb")
                    for oc in range(2):
                        acc = ps_d.tile([128, 512], F32, tag="o")
                        for t in range(2):
                            nc.tensor.matmul(acc[:], yP[t][:, ms],
                                             wpP_sb[t][:,
                                                       oc * 512:(oc + 1) * 512],
                                             start=(t == 0), stop=(t == 1))
                        nc.vector.tensor_copy(
                            o_sb[:, oc * 512:(oc + 1) * 512], acc[:])
                    # the last i-block's stores go on the (idle) scalar queue
                    eng = nc.scalar if ib == 0 else nc.sync
                    eng.dma_start(out[ms, :], o_sb[:])

    nc.compile()
    return nc


def _host_inputs(x, w_attn, w_proj):
    """Build the 8 per-core input maps."""
    bf = ml_dtypes.bfloat16
    inv_freq = 1.0 / (10000.0 ** (np.arange(0, D_HEAD, 2, dtype=np.float32)
                                  / D_HEAD))
    t = np.arange(T, dtype=np.float32)
    freqs = np.einsum('i,j->ij', t, inv_freq)          # [T, 32]
    cos64 = np.cos(np.concatenate([freqs, freqs], 1)).T  # [64, T]
    sin64 = np.sin(np.concatenate([freqs, freqs], 1)).T
    cosT = np.concatenate([cos64, cos64], 0).astype(bf)  # [128, T]
    sinT = np.concatenate([sin64, sin64], 0).astype(bf)

    # rot matrix: accr = M @ S with M[d,e] = -1 if e==d+32 (d%64<32),
    # +1 if e==d-32 (d%64>=32); matmul computes lhsT.T @ rhs -> pass M.T
    M = np.zeros((128, 128), np.float32)
    for d in range(128):
        base = (d // 64) * 64
        r = d % 64
        if r < 32:
            M[d, base + r + 32] = -1.0
        else:
            M[d, base + r - 32] = 1.0
    permT = M.T.astype(bf).copy()

    tri = (np.arange(128)[:, None] <= np.arange(128)[None, :]).astype(bf)
    rsel32 = np.zeros((128, 16 * 32), np.float32)
    rselT32 = np.zeros((32, 16 * 128), np.float32)
    for chm in range(16):
        ch, mt = chm // 4, chm % 4
        for half in range(2):
            r = ch * 8 + mt * 2 + half
            ps = slice(half * 64, half * 64 + 64)
            rsel32[ps, chm * 32 + r] = 1.0
            rselT32[r, chm * 128 + half * 64:chm * 128 + half * 64 + 64] = 1.0
    # selS: cols 0-63 pick partition 64 (even-head sums row), cols 64-127
    # pick partition 63 (odd-head sums row)
    selS = np.zeros((128, 128), np.float32)
    selS[64, 0:64] = 1.0
    selS[63, 64:128] = 1.0
    onescol = np.ones((128, HL * NTT), np.float32)

    wq = w_attn[:D_MODEL]          # [1024, 1024] rows: head h = 64h..64h+63
    wk = w_attn[D_MODEL:2 * D_MODEL]
    wv_full = w_attn[2 * D_MODEL:]

    in_maps = []
    for c in range(N_CORES):
        b, hg = c // 4, c % 4
        hs = slice(hg * 4 * D_HEAD, (hg * 4 + 4) * D_HEAD)   # 256 rows
        wqk_c = np.concatenate([wq[hs], wk[hs]], 0).T        # [1024, 512]
        wv_c = wv_full[hs].T                                 # [1024, 256]
        wp_c = [w_proj[:, (hg * 4 + j) * D_HEAD:(hg * 4 + j + 1) * D_HEAD].T
                for j in range(HL)]                          # 4x[64,1024]
        wpP_c = np.stack([np.concatenate([wp_c[0], wp_c[1]], 0),
                          np.concatenate([wp_c[2], wp_c[3]], 0)])
        in_maps.append({
            "xT": np.ascontiguousarray(x[b].T.astype(bf)),
            "wqk": np.ascontiguousarray(wqk_c.astype(bf)),
            "wv": np.ascontiguousarray(wv_c.astype(bf)),
            "wpP": np.ascontiguousarray(wpP_c.astype(bf)),
            "cosT": cosT, "sinT": sinT, "permT": permT, "trimask": tri,
            "rsel32": rsel32.astype(bf), "rselT32": rselT32.astype(bf),
            "selS": selS.astype(bf), "onescol": onescol.astype(bf),
        })
    return in_maps


def kernel(x, w_attn, w_proj, _want_results=False):
    x = np.asarray(x, dtype=np.float32)
    w_attn = np.asarray(w_attn, dtype=np.float32)
    w_proj = np.asarray(w_proj, dtype=np.float32)

    if "nc" not in _cached:
        _cached["nc"] = _build()
    nc = _cached["nc"]

    in_maps = _host_inputs(x, w_attn, w_proj)
    res = run_bass_kernel_spmd(nc, in_maps, list(range(N_CORES)))

    full = np.zeros((B, T, D_MODEL), np.float32)
    for c in range(N_CORES):
        full[c // 4] += np.asarray(res.results[c]["out"], dtype=np.float32)
    if _want_results:
        return full, res
    return full
